# revision 28
# baseline (speedup 1.0000x reference)
"""DRNN-Char (4-layer dilated QRNN + decoder) Trainium2 kernel, v12.

Sharding: data-parallel over batch. 16 rows / 8 cores = 2 rows per core.
~276us HW (vs 1050us for the staged baseline in this environment), rel err 7.5e-3.

Design, all rates hardware-measured:
- Layer 0 is a host-side table lookup (gates depend only on the input token);
  tables arrive per (row, h-chunk) as [128, T] bf16 so L0 is one merged scan +
  one TT per (r,h). fp8 tables fail numerically (4.3e-2).
- All elementwise state is bf16: tensor_tensor_scan keeps an fp32 internal
  carry regardless of operand dtype, and scan rate is dtype-independent
  (~2.15ns/elem), so bf16 costs no accuracy in the carry and speeds the
  surrounding DVE ops into 2x/4x modes (TT 0.59ns/elem, TS 0.33ns/elem).
- DVE owns the whole per-unit chain: fm1 = f-1 (TS 4x), zpt = fm1*ztneg
  (TT 2x), ONE merged [128,T] scan per unit (f is zeroed at dilation
  subsequence starts via a strided TS, making the carry reset exact:
  c = 0*c_prev + zpt), h = o*cc (TT). Offloading any link of this chain to
  GpSimd (stock TT ~4us/[128,2048], shares an SBUF port with DVE) measured
  20-60us WORSE: cross-engine hops serialize the scan pipeline.
- z matmul bf16 (fp8 z measured 5.6e-2 rel err - x8 quantization is
  first-order in z); f/o fp8e4 DoubleRow. Dilation reorder rides on stride-2
  matmul rhs APs (measured: no matmul stride penalty).
- fp8 x-copies are ACT Copy-with-scale (GpSimd tensor_scalar measured 29us/
  [128,2048]; DVE TS 1.2us; ACT 2.0us) emitted AFTER the next unit's gate
  ACTs so ACT never stalls on the current unit's DVE chain; one convert per
  layer-row goes on DVE to balance the two engines.
- Const weight DMAs are emitted after the L0 table DMAs (they delayed the
  first so0 chunks ~10us); tables spread over sync/scalar/gpsimd queues;
  decoder output DMAs round-robin over the three queues.
- decb is added on the host (a [V] broadcast over the f32 output).
"""

import numpy as np
import ml_dtypes

EMB = 256
HID = 512
LAYERS = 4
VOCAB = 256
B = 16
T = 2048
NCORES = 8
BC = B // NCORES
HCH = HID // 128

SW = 32.0                      # fp8 weight scale
SX = [32.0, 128.0, 256.0]      # fp8 x scale for h0,h1,h2 (inputs of L1..L3)

_cache = {}


def _build():
    if "nc" in _cache:
        return _cache["nc"]

    import concourse.bass as bass
    import concourse.mybir as mybir
    import concourse.tile as tile
    from concourse import bacc

    f32 = mybir.dt.float32
    bf16 = mybir.dt.bfloat16
    fp8 = mybir.dt.float8e4
    SIG = mybir.ActivationFunctionType.Sigmoid
    TANH = mybir.ActivationFunctionType.Tanh
    COPY = mybir.ActivationFunctionType.Copy
    MULT = mybir.AluOpType.mult
    ADD = mybir.AluOpType.add
    DR = mybir.MatmulPerfMode.DoubleRow

    nc = bacc.Bacc(
        "TRN2",
        target_bir_lowering=False,
        debug=False,
        enable_asserts=False,
        num_devices=NCORES,
    )

    # ---- DRAM inputs (host-prepped, per core) ----
    zp0_d = nc.dram_tensor("zp0", [BC, 4, 128, T], bf16, kind="ExternalInput").ap()
    f0_d = nc.dram_tensor("f0", [BC, 4, 128, T], bf16, kind="ExternalInput").ap()
    so0_d = nc.dram_tensor("so0", [BC, 4, 128, T], bf16, kind="ExternalInput").ap()
    wz_d = nc.dram_tensor("wz", [3, 128, 4, 512], bf16, kind="ExternalInput").ap()
    wfo_d = nc.dram_tensor("wfo", [3, 128, 4, 1024], fp8, kind="ExternalInput").ap()
    wd_d = nc.dram_tensor("wd", [128, 4, VOCAB], bf16, kind="ExternalInput").ap()
    bias_d = nc.dram_tensor("bias", [128, 3, 12], f32, kind="ExternalInput").ap()
    out_d = nc.dram_tensor("out", [BC, T, VOCAB], f32, kind="ExternalOutput").ap()

    with tile.TileContext(nc) as tc:
        with (
            tc.tile_pool(name="consts", bufs=1) as consts,
            tc.tile_pool(name="acts", bufs=1) as acts,
            tc.tile_pool(name="l0t", bufs=2) as l0t,
            tc.tile_pool(name="stage", bufs=2) as stage,
            tc.tile_pool(name="ccl0", bufs=2) as ccl0,
            tc.tile_pool(name="ccp", bufs=2) as ccp,
            tc.tile_pool(name="outs", bufs=5) as outs,
            tc.tile_pool(name="psum", bufs=4, space="PSUM") as psum,
        ):
            # ---- resident tiles ----
            wz_sb = [consts.tile([128, 4, 512], bf16, tag=f"wz{i}", name=f"wz{i}") for i in range(3)]
            wfo_sb = [consts.tile([128, 4, 1024], fp8, tag=f"wfo{i}", name=f"wfo{i}") for i in range(3)]
            wd = consts.tile([128, 4, VOCAB], bf16, tag="wd", name="wd")
            bias = consts.tile([128, 3, 12], f32, tag="bias", name="bias")

            xbuf = [[acts.tile([128, 4, T], bf16, tag=f"x{r}_{p}", name=f"x{r}_{p}")
                     for p in range(2)] for r in range(BC)]
            x8 = [[acts.tile([128, 4, T], fp8, tag=f"x8{r}_{p}", name=f"x8{r}_{p}")
                   for p in range(2)] for r in range(BC)]

            # ---- layer 0: scan over host-gathered tables ----
            # (const weight DMAs are emitted AFTER the table DMAs so the
            # gpsimd queue doesn't delay the first so0 chunks by ~10us;
            # weights are only needed when L1 starts)
            # tables arrive per (row, h-chunk) as [128, T]: one merged scan and
            # one TT per (r,h), no carry chaining across chunks
            for r in range(BC):
                for h in range(HCH):
                    zp = l0t.tile([128, T], bf16, tag="zp", name="zp")
                    f0t = l0t.tile([128, T], bf16, tag="f0", name="f0")
                    so0t = l0t.tile([128, T], bf16, tag="so0", name="so0")
                    nc.sync.dma_start(zp[:], zp0_d[r, h])
                    nc.scalar.dma_start(f0t[:], f0_d[r, h])
                    nc.gpsimd.dma_start(so0t[:], so0_d[r, h])
                    cq = ccl0.tile([128, T], bf16, tag="cc0", name="cc0")
                    nc.vector.tensor_tensor_scan(
                        cq[:], f0t[:], zp[:],
                        initial=0.0, op0=MULT, op1=ADD,
                    )
                    nc.vector.tensor_tensor(xbuf[r][0][:, h, :], so0t[:], cq[:], MULT)
                    nc.scalar.activation(
                        x8[r][0][:, h, :], xbuf[r][0][:, h, :], COPY,
                        bias=0.0, scale=SX[0],
                    )

            # ---- const DMAs (behind the table DMAs on the gpsimd queue) ----
            for i in range(3):
                nc.gpsimd.dma_start(wz_sb[i][:], wz_d[i])
                nc.gpsimd.dma_start(wfo_sb[i][:], wfo_d[i])
            nc.gpsimd.dma_start(wd[:], wd_d[:])
            nc.gpsimd.dma_start(bias[:], bias_d[:])

            # ---- layers 1..3 ----
            pend_act = []   # (out, in, scale) converts for the ACT queue
            pend_dve = []   # converts for the DVE queue

            def flush_act():
                while pend_act:
                    o_ap, i_ap, s = pend_act.pop(0)
                    nc.scalar.activation(o_ap, i_ap, COPY, bias=0.0, scale=s)

            def flush_dve():
                while pend_dve:
                    o_ap, i_ap, s = pend_dve.pop(0)
                    nc.vector.tensor_scalar_mul(o_ap, i_ap, s)

            cur = 0
            for li in (1, 2, 3):
                idx = li - 1
                rho = 2 ** li
                ascale = 1.0 / (SW * SX[idx])
                nxt = 1 - cur
                for r in range(BC):
                    xin, x8in = xbuf[r][cur], x8[r][cur]
                    xout, x8out = xbuf[r][nxt], x8[r][nxt]
                    for h in range(HCH):
                        # z gate: bf16; ztneg = tanh(-(pre + bz))
                        zt = stage.tile([128, T], bf16, tag="zt", name="zt")
                        for pb in range(2):
                            ps = psum.tile([128, 1024], f32, tag="ps", name="ps")
                            for k in range(4):
                                for u in range(2):
                                    nc.tensor.matmul(
                                        ps[:, u * 512 : (u + 1) * 512],
                                        lhsT=wz_sb[idx][:, k, h * 128 : (h + 1) * 128],
                                        rhs=xin[:, k, pb + u * 1024 : pb + u * 1024 + 1023 : 2],
                                        start=(k == 0),
                                        stop=(k == 3),
                                    )
                            nc.scalar.activation(
                                zt[:, pb * 1024 : (pb + 1) * 1024], ps[:], TANH,
                                bias=bias[:, idx, h : h + 1], scale=-1.0,
                            )
                        # f,o gates: fp8 DoubleRow, bf16 out
                        gts = {"z": zt}
                        for g, gname in ((0, "f"), (1, "o")):
                            gt = stage.tile([128, T], bf16, tag=gname, name=gname)
                            for pb in range(2):
                                ps = psum.tile([128, 1024], f32, tag="ps", name="ps")
                                for kp in range(2):
                                    for u in range(2):
                                        nc.tensor.matmul(
                                            ps[:, u * 512 : (u + 1) * 512],
                                            lhsT=wfo_sb[idx][:, 2 * kp : 2 * kp + 2, g * 512 + h * 128 : g * 512 + (h + 1) * 128],
                                            rhs=x8in[:, 2 * kp : 2 * kp + 2, pb + u * 1024 : pb + u * 1024 + 1023 : 2],
                                            start=(kp == 0),
                                            stop=(kp == 1),
                                            perf_mode=DR,
                                        )
                                nc.scalar.activation(
                                    gt[:, pb * 1024 : (pb + 1) * 1024], ps[:], SIG,
                                    bias=bias[:, idx, (g + 1) * 4 + h : (g + 1) * 4 + h + 1],
                                    scale=ascale,
                                )
                            gts[gname] = gt
                        flush_act()
                        # fm1 = f - 1 (DVE TS, 4x); zpt = fm1 * ztneg (DVE TT, 2x)
                        fm1 = ccp.tile([128, T], bf16, tag="fm1", name="fm1")
                        nc.vector.tensor_scalar_add(fm1[:], gts["f"][:], -1.0)
                        flush_dve()
                        zpt = ccp.tile([128, T], bf16, tag="zp", name="zp")
                        nc.vector.tensor_tensor(zpt[:], fm1[:], gts["z"][:], MULT)
                        # zero f at subsequence starts -> merged scan resets exactly
                        # (c = 0*c_prev + zpt); gpsimd memset — the gpsimd queue
                        # carries nothing else during the layer phase
                        if rho > 1:
                            nc.gpsimd.memset(gts["f"][:, 0 : T : T // rho], 0.0)
                        cc = ccp.tile([128, T], bf16, tag="cc", name="cc")
                        nc.vector.tensor_tensor_scan(
                            cc[:], gts["f"][:], zpt[:],
                            initial=0.0, op0=MULT, op1=ADD,
                        )
                        nc.vector.tensor_tensor(xout[:, h, :], gts["o"][:], cc[:], MULT)
                        if li < 3:
                            pend_act.append((x8out[:, h, :], xout[:, h, :], SX[li]))
                cur = nxt
            flush_act()
            flush_dve()

            # ---- decoder (h3 in dilation-8 order; scatter rows on DMA out) ----
            # psum-drain copies alternate ACT/DVE so consecutive tiles drain on
            # two engines; output DMA issues stay off the scalar (ACT) queue
            dma_engines = [nc.sync, nc.gpsimd]
            di = 0
            for r in range(BC):
                xin = xbuf[r][cur]
                for mt in range(T // 128):
                    ps = psum.tile([128, 1024], f32, tag="ps", name="ps")
                    for k in range(4):
                        nc.tensor.matmul(
                            ps[:, 0:VOCAB],
                            lhsT=xin[:, k, mt * 128 : (mt + 1) * 128],
                            rhs=wd[:, k, :],
                            start=(k == 0),
                            stop=(k == 3),
                        )
                    ot = outs.tile([128, VOCAB], f32, tag="ot", name="ot")
                    if mt % 2 == 0:
                        nc.scalar.activation(ot[:], ps[:, 0:VOCAB], COPY, bias=0.0, scale=1.0)
                    else:
                        nc.vector.tensor_copy(ot[:], ps[:, 0:VOCAB])
                    # dilation-8 index i = j*256 + q -> t = 8q + j
                    t0 = 1024 * (mt % 2) + mt // 2
                    dma_engines[di % len(dma_engines)].dma_start(
                        out_d[r, t0 : t0 + 1017 : 8, :], ot[:]
                    )
                    di += 1

    nc.compile()
    _cache["nc"] = nc
    return nc


def _prep_inputs(inputs):
    bf = ml_dtypes.bfloat16
    f8 = ml_dtypes.float8_e4m3fn
    x = np.asarray(inputs["x"]).astype(np.int64)
    emb = np.asarray(inputs["emb"], dtype=np.float32)
    Ws = [np.asarray(inputs[f"W{i}"], dtype=np.float32) for i in range(LAYERS)]
    bs = [np.asarray(inputs[f"b{i}"], dtype=np.float32) for i in range(LAYERS)]
    decW = np.asarray(inputs["decW"], dtype=np.float32)

    pre0 = emb @ Ws[0] + bs[0]          # [VOCAB, 3H]
    zt0 = np.tanh(pre0[:, :HID])
    f0 = 1.0 / (1.0 + np.exp(-pre0[:, HID : 2 * HID]))
    so0 = 1.0 / (1.0 + np.exp(-pre0[:, 2 * HID :]))
    f0 = f0.astype(bf).astype(np.float32)
    zp0 = ((1.0 - f0) * zt0).astype(bf)
    so0b = so0.astype(bf)

    def table_arrange(tab, idx):
        g = tab[idx]                                  # [T, 512]
        return np.ascontiguousarray(g.T.reshape(4, 128, T))  # [4h, 128, T]

    wz = np.stack(
        [np.ascontiguousarray(Ws[i][:, :HID].reshape(4, 128, 512).transpose(1, 0, 2)).astype(bf) for i in range(1, 4)]
    )
    wfo = np.stack(
        [
            np.ascontiguousarray((Ws[i][:, HID:] * SW).reshape(4, 128, 1024).transpose(1, 0, 2)).astype(f8)
            for i in range(1, 4)
        ]
    )
    wdt = np.ascontiguousarray(decW.reshape(4, 128, VOCAB).transpose(1, 0, 2)).astype(bf)

    bias = np.zeros((128, 3, 12), np.float32)
    for i in range(1, 4):
        bb = bs[i].reshape(3, 4, 128)  # [gate, h, p]
        bias[:, i - 1, 0:4] = -bb[0].T
        bias[:, i - 1, 4:8] = bb[1].T
        bias[:, i - 1, 8:12] = bb[2].T

    in_maps = []
    for c in range(NCORES):
        zp_r = np.stack([table_arrange(zp0, x[BC * c + r]) for r in range(BC)])
        f_r = np.stack([table_arrange(f0.astype(bf), x[BC * c + r]) for r in range(BC)])
        so_r = np.stack([table_arrange(so0b, x[BC * c + r]) for r in range(BC)])
        in_maps.append(
            {
                "zp0": zp_r,
                "f0": f_r,
                "so0": so_r,
                "wz": wz,
                "wfo": wfo,
                "wd": wdt,
                "bias": bias,
            }
        )
    return in_maps


def _unpermute(res, decb):
    out = np.empty((B, T, VOCAB), np.float32)
    for c in range(NCORES):
        out[BC * c : BC * (c + 1)] = res[c]["out"]
    out += decb.reshape(1, 1, VOCAB)
    return out


def kernel(**inputs) -> np.ndarray:
    from concourse.bass_utils import run_bass_kernel_spmd

    try:
        import jax, tempfile, os

        jax.config.update(
            "jax_compilation_cache_dir",
            os.environ.get("JAX_COMPILATION_CACHE_DIR")
            or os.path.join(tempfile.gettempdir(), "bass_jax_cache"),
        )
    except Exception:
        pass

    nc = _build()
    in_maps = _prep_inputs(inputs)
    res = run_bass_kernel_spmd(nc, in_maps, list(range(NCORES)))
    decb = np.asarray(inputs["decb"], dtype=np.float32)
    return _unpermute(res.results, decb)


# revision 29
# speedup vs baseline: 1.0363x; 1.0363x over previous
"""DRNN-Char (4-layer dilated QRNN + decoder) Trainium2 kernel, v12.

Sharding: data-parallel over batch. 16 rows / 8 cores = 2 rows per core.
~276us HW (vs 1050us for the staged baseline in this environment), rel err 7.5e-3.

Design, all rates hardware-measured:
- Layer 0 is a host-side table lookup (gates depend only on the input token);
  tables arrive per (row, h-chunk) as [128, T] bf16 so L0 is one merged scan +
  one TT per (r,h). fp8 tables fail numerically (4.3e-2).
- All elementwise state is bf16: tensor_tensor_scan keeps an fp32 internal
  carry regardless of operand dtype, and scan rate is dtype-independent
  (~2.15ns/elem), so bf16 costs no accuracy in the carry and speeds the
  surrounding DVE ops into 2x/4x modes (TT 0.59ns/elem, TS 0.33ns/elem).
- DVE owns the whole per-unit chain: fm1 = f-1 (TS 4x), zpt = fm1*ztneg
  (TT 2x), ONE merged [128,T] scan per unit (f is zeroed at dilation
  subsequence starts via a strided TS, making the carry reset exact:
  c = 0*c_prev + zpt), h = o*cc (TT). Offloading any link of this chain to
  GpSimd (stock TT ~4us/[128,2048], shares an SBUF port with DVE) measured
  20-60us WORSE: cross-engine hops serialize the scan pipeline.
- z matmul bf16 (fp8 z measured 5.6e-2 rel err - x8 quantization is
  first-order in z); f/o fp8e4 DoubleRow. Dilation reorder rides on stride-2
  matmul rhs APs (measured: no matmul stride penalty).
- fp8 x-copies are ACT Copy-with-scale (GpSimd tensor_scalar measured 29us/
  [128,2048]; DVE TS 1.2us; ACT 2.0us) emitted AFTER the next unit's gate
  ACTs so ACT never stalls on the current unit's DVE chain; one convert per
  layer-row goes on DVE to balance the two engines.
- Const weight DMAs are emitted after the L0 table DMAs (they delayed the
  first so0 chunks ~10us); tables spread over sync/scalar/gpsimd queues;
  decoder output DMAs round-robin over the three queues.
- decb is added on the host (a [V] broadcast over the f32 output).
"""

import numpy as np
import ml_dtypes

EMB = 256
HID = 512
LAYERS = 4
VOCAB = 256
B = 16
T = 2048
NCORES = 8
BC = B // NCORES
HCH = HID // 128

SW = 32.0                      # fp8 weight scale
SX = [32.0, 128.0, 256.0]      # fp8 x scale for h0,h1,h2 (inputs of L1..L3)

_cache = {}


def _build():
    if "nc" in _cache:
        return _cache["nc"]

    import concourse.bass as bass
    import concourse.mybir as mybir
    import concourse.tile as tile
    from concourse import bacc

    f32 = mybir.dt.float32
    bf16 = mybir.dt.bfloat16
    fp8 = mybir.dt.float8e4
    SIG = mybir.ActivationFunctionType.Sigmoid
    TANH = mybir.ActivationFunctionType.Tanh
    COPY = mybir.ActivationFunctionType.Copy
    MULT = mybir.AluOpType.mult
    ADD = mybir.AluOpType.add
    DR = mybir.MatmulPerfMode.DoubleRow

    nc = bacc.Bacc(
        "TRN2",
        target_bir_lowering=False,
        debug=False,
        enable_asserts=False,
        num_devices=NCORES,
    )

    # ---- DRAM inputs (host-prepped, per core) ----
    zp0_d = nc.dram_tensor("zp0", [BC, 4, 128, T], bf16, kind="ExternalInput").ap()
    f0_d = nc.dram_tensor("f0", [BC, 4, 128, T], bf16, kind="ExternalInput").ap()
    so0_d = nc.dram_tensor("so0", [BC, 4, 128, T], bf16, kind="ExternalInput").ap()
    wz_d = nc.dram_tensor("wz", [3, 128, 4, 512], bf16, kind="ExternalInput").ap()
    wfo_d = nc.dram_tensor("wfo", [3, 128, 4, 1024], fp8, kind="ExternalInput").ap()
    wd_d = nc.dram_tensor("wd", [128, 4, VOCAB], bf16, kind="ExternalInput").ap()
    bias_d = nc.dram_tensor("bias", [128, 3, 12], f32, kind="ExternalInput").ap()
    out_d = nc.dram_tensor("out", [BC, T, VOCAB], f32, kind="ExternalOutput").ap()

    with tile.TileContext(nc) as tc:
        with (
            tc.tile_pool(name="consts", bufs=1) as consts,
            tc.tile_pool(name="acts", bufs=1) as acts,
            tc.tile_pool(name="l0t", bufs=2) as l0t,
            tc.tile_pool(name="stage", bufs=2) as stage,
            tc.tile_pool(name="ccl0", bufs=2) as ccl0,
            tc.tile_pool(name="ccp", bufs=2) as ccp,
            tc.tile_pool(name="outs", bufs=5) as outs,
            tc.tile_pool(name="psum", bufs=4, space="PSUM") as psum,
        ):
            # ---- resident tiles ----
            wz_sb = [consts.tile([128, 4, 512], bf16, tag=f"wz{i}", name=f"wz{i}") for i in range(3)]
            wfo_sb = [consts.tile([128, 4, 1024], fp8, tag=f"wfo{i}", name=f"wfo{i}") for i in range(3)]
            wd = consts.tile([128, 4, VOCAB], bf16, tag="wd", name="wd")
            bias = consts.tile([128, 3, 12], f32, tag="bias", name="bias")

            xbuf = [[acts.tile([128, 4, T], bf16, tag=f"x{r}_{p}", name=f"x{r}_{p}")
                     for p in range(2)] for r in range(BC)]
            x8 = [[acts.tile([128, 4, T], fp8, tag=f"x8{r}_{p}", name=f"x8{r}_{p}")
                   for p in range(2)] for r in range(BC)]

            # ---- layer 0: scan over host-gathered tables ----
            # (const weight DMAs are emitted AFTER the table DMAs so the
            # gpsimd queue doesn't delay the first so0 chunks by ~10us;
            # weights are only needed when L1 starts)
            # tables arrive per (row, h-chunk) as [128, T]: one merged scan and
            # one TT per (r,h), no carry chaining across chunks
            for r in range(BC):
                for h in range(HCH):
                    zp = l0t.tile([128, T], bf16, tag="zp", name="zp")
                    f0t = l0t.tile([128, T], bf16, tag="f0", name="f0")
                    so0t = l0t.tile([128, T], bf16, tag="so0", name="so0")
                    nc.sync.dma_start(zp[:], zp0_d[r, h])
                    nc.scalar.dma_start(f0t[:], f0_d[r, h])
                    nc.gpsimd.dma_start(so0t[:], so0_d[r, h])
                    cq = ccl0.tile([128, T], bf16, tag="cc0", name="cc0")
                    nc.vector.tensor_tensor_scan(
                        cq[:], f0t[:], zp[:],
                        initial=0.0, op0=MULT, op1=ADD,
                    )
                    nc.vector.tensor_tensor(xbuf[r][0][:, h, :], so0t[:], cq[:], MULT)
                    nc.scalar.activation(
                        x8[r][0][:, h, :], xbuf[r][0][:, h, :], COPY,
                        bias=0.0, scale=SX[0],
                    )

            # ---- const DMAs (behind the table DMAs on the gpsimd queue) ----
            for i in range(3):
                nc.gpsimd.dma_start(wz_sb[i][:], wz_d[i])
                nc.gpsimd.dma_start(wfo_sb[i][:], wfo_d[i])
            nc.gpsimd.dma_start(wd[:], wd_d[:])
            nc.gpsimd.dma_start(bias[:], bias_d[:])

            # ---- layers 1..3 ----
            pend_act = []   # (out, in, scale) converts for the ACT queue
            pend_dve = []   # converts for the DVE queue

            def flush_act():
                while pend_act:
                    o_ap, i_ap, s = pend_act.pop(0)
                    nc.scalar.activation(o_ap, i_ap, COPY, bias=0.0, scale=s)

            def flush_dve():
                while pend_dve:
                    o_ap, i_ap, s = pend_dve.pop(0)
                    nc.vector.tensor_scalar_mul(o_ap, i_ap, s)

            cur = 0
            for li in (1, 2, 3):
                idx = li - 1
                rho = 2 ** li
                ascale = 1.0 / (SW * SX[idx])
                nxt = 1 - cur
                for r in range(BC):
                    xin, x8in = xbuf[r][cur], x8[r][cur]
                    xout, x8out = xbuf[r][nxt], x8[r][nxt]
                    for h in range(HCH):
                        # z gate: bf16; ztneg = tanh(-(pre + bz))
                        zt = stage.tile([128, T], bf16, tag="zt", name="zt")
                        for pb in range(2):
                            ps = psum.tile([128, 1024], f32, tag="ps", name="ps")
                            for k in range(4):
                                for u in range(2):
                                    nc.tensor.matmul(
                                        ps[:, u * 512 : (u + 1) * 512],
                                        lhsT=wz_sb[idx][:, k, h * 128 : (h + 1) * 128],
                                        rhs=xin[:, k, pb + u * 1024 : pb + u * 1024 + 1023 : 2],
                                        start=(k == 0),
                                        stop=(k == 3),
                                    )
                            nc.scalar.activation(
                                zt[:, pb * 1024 : (pb + 1) * 1024], ps[:], TANH,
                                bias=bias[:, idx, h : h + 1], scale=-1.0,
                            )
                        # f,o gates: fp8 DoubleRow, bf16 out
                        gts = {"z": zt}
                        for g, gname in ((0, "f"), (1, "o")):
                            gt = stage.tile([128, T], bf16, tag=gname, name=gname)
                            for pb in range(2):
                                ps = psum.tile([128, 1024], f32, tag="ps", name="ps")
                                for kp in range(2):
                                    for u in range(2):
                                        nc.tensor.matmul(
                                            ps[:, u * 512 : (u + 1) * 512],
                                            lhsT=wfo_sb[idx][:, 2 * kp : 2 * kp + 2, g * 512 + h * 128 : g * 512 + (h + 1) * 128],
                                            rhs=x8in[:, 2 * kp : 2 * kp + 2, pb + u * 1024 : pb + u * 1024 + 1023 : 2],
                                            start=(kp == 0),
                                            stop=(kp == 1),
                                            perf_mode=DR,
                                        )
                                nc.scalar.activation(
                                    gt[:, pb * 1024 : (pb + 1) * 1024], ps[:], SIG,
                                    bias=bias[:, idx, (g + 1) * 4 + h : (g + 1) * 4 + h + 1],
                                    scale=ascale,
                                )
                            gts[gname] = gt
                        flush_act()
                        # fm1 = f - 1 (DVE TS, 4x); zpt = fm1 * ztneg (DVE TT, 2x)
                        fm1 = ccp.tile([128, T], bf16, tag="fm1", name="fm1")
                        nc.vector.tensor_scalar_add(fm1[:], gts["f"][:], -1.0)
                        flush_dve()
                        zpt = ccp.tile([128, T], bf16, tag="zp", name="zp")
                        nc.vector.tensor_tensor(zpt[:], fm1[:], gts["z"][:], MULT)
                        # zero f at subsequence starts -> merged scan resets exactly
                        # (c = 0*c_prev + zpt); gpsimd memset — the gpsimd queue
                        # carries nothing else during the layer phase
                        if rho > 1:
                            nc.gpsimd.memset(gts["f"][:, 0 : T : T // rho], 0.0)
                        cc = ccp.tile([128, T], bf16, tag="cc", name="cc")
                        nc.vector.tensor_tensor_scan(
                            cc[:], gts["f"][:], zpt[:],
                            initial=0.0, op0=MULT, op1=ADD,
                        )
                        nc.vector.tensor_tensor(xout[:, h, :], gts["o"][:], cc[:], MULT)
                        if li < 3:
                            pend_act.append((x8out[:, h, :], xout[:, h, :], SX[li]))
                cur = nxt
            flush_act()
            flush_dve()

            # ---- decoder (h3 in dilation-8 order; scatter rows on DMA out) ----
            # all psum-drain copies on ACT: DVE-queued copies for row 0 would
            # sit behind row 1's L3 scan work and break the dec(r0) overlap
            dma_engines = [nc.sync, nc.gpsimd, nc.scalar]
            di = 0
            for r in range(BC):
                xin = xbuf[r][cur]
                for mt in range(T // 128):
                    ps = psum.tile([128, 1024], f32, tag="ps", name="ps")
                    for k in range(4):
                        nc.tensor.matmul(
                            ps[:, 0:VOCAB],
                            lhsT=xin[:, k, mt * 128 : (mt + 1) * 128],
                            rhs=wd[:, k, :],
                            start=(k == 0),
                            stop=(k == 3),
                        )
                    ot = outs.tile([128, VOCAB], f32, tag="ot", name="ot")
                    nc.scalar.activation(ot[:], ps[:, 0:VOCAB], COPY, bias=0.0, scale=1.0)
                    # dilation-8 index i = j*256 + q -> t = 8q + j
                    t0 = 1024 * (mt % 2) + mt // 2
                    dma_engines[di % len(dma_engines)].dma_start(
                        out_d[r, t0 : t0 + 1017 : 8, :], ot[:]
                    )
                    di += 1

    nc.compile()
    _cache["nc"] = nc
    return nc


def _prep_inputs(inputs):
    bf = ml_dtypes.bfloat16
    f8 = ml_dtypes.float8_e4m3fn
    x = np.asarray(inputs["x"]).astype(np.int64)
    emb = np.asarray(inputs["emb"], dtype=np.float32)
    Ws = [np.asarray(inputs[f"W{i}"], dtype=np.float32) for i in range(LAYERS)]
    bs = [np.asarray(inputs[f"b{i}"], dtype=np.float32) for i in range(LAYERS)]
    decW = np.asarray(inputs["decW"], dtype=np.float32)

    pre0 = emb @ Ws[0] + bs[0]          # [VOCAB, 3H]
    zt0 = np.tanh(pre0[:, :HID])
    f0 = 1.0 / (1.0 + np.exp(-pre0[:, HID : 2 * HID]))
    so0 = 1.0 / (1.0 + np.exp(-pre0[:, 2 * HID :]))
    f0 = f0.astype(bf).astype(np.float32)
    zp0 = ((1.0 - f0) * zt0).astype(bf)
    so0b = so0.astype(bf)

    def table_arrange(tab, idx):
        g = tab[idx]                                  # [T, 512]
        return np.ascontiguousarray(g.T.reshape(4, 128, T))  # [4h, 128, T]

    wz = np.stack(
        [np.ascontiguousarray(Ws[i][:, :HID].reshape(4, 128, 512).transpose(1, 0, 2)).astype(bf) for i in range(1, 4)]
    )
    wfo = np.stack(
        [
            np.ascontiguousarray((Ws[i][:, HID:] * SW).reshape(4, 128, 1024).transpose(1, 0, 2)).astype(f8)
            for i in range(1, 4)
        ]
    )
    wdt = np.ascontiguousarray(decW.reshape(4, 128, VOCAB).transpose(1, 0, 2)).astype(bf)

    bias = np.zeros((128, 3, 12), np.float32)
    for i in range(1, 4):
        bb = bs[i].reshape(3, 4, 128)  # [gate, h, p]
        bias[:, i - 1, 0:4] = -bb[0].T
        bias[:, i - 1, 4:8] = bb[1].T
        bias[:, i - 1, 8:12] = bb[2].T

    in_maps = []
    for c in range(NCORES):
        zp_r = np.stack([table_arrange(zp0, x[BC * c + r]) for r in range(BC)])
        f_r = np.stack([table_arrange(f0.astype(bf), x[BC * c + r]) for r in range(BC)])
        so_r = np.stack([table_arrange(so0b, x[BC * c + r]) for r in range(BC)])
        in_maps.append(
            {
                "zp0": zp_r,
                "f0": f_r,
                "so0": so_r,
                "wz": wz,
                "wfo": wfo,
                "wd": wdt,
                "bias": bias,
            }
        )
    return in_maps


def _unpermute(res, decb):
    out = np.empty((B, T, VOCAB), np.float32)
    for c in range(NCORES):
        out[BC * c : BC * (c + 1)] = res[c]["out"]
    out += decb.reshape(1, 1, VOCAB)
    return out


def kernel(**inputs) -> np.ndarray:
    from concourse.bass_utils import run_bass_kernel_spmd

    try:
        import jax, tempfile, os

        jax.config.update(
            "jax_compilation_cache_dir",
            os.environ.get("JAX_COMPILATION_CACHE_DIR")
            or os.path.join(tempfile.gettempdir(), "bass_jax_cache"),
        )
    except Exception:
        pass

    nc = _build()
    in_maps = _prep_inputs(inputs)
    res = run_bass_kernel_spmd(nc, in_maps, list(range(NCORES)))
    decb = np.asarray(inputs["decb"], dtype=np.float32)
    return _unpermute(res.results, decb)
